# revision 57
# baseline (speedup 1.0000x reference)
"""Trainium2 Bass kernel for nn_MemTransformerLM (Transformer-XL layer).

Sharding (8 cores): batch (4) x head-half (2). Core c handles batch b = c//2
and heads [hh*8, hh*8+8), hh = c%2, for all 1024 queries. After o_proj a
2-rank ReduceScatter over core pairs (2b, 2b+1) splits tokens for the FFN:
even core keeps tokens [0,512), odd [512,1024).

Pipeline-oriented rewrite of the original phase-serial kernel:
 - fp32->bf16 casts folded into gpsimd (SWDGE) DMA loads.
 - input transposes on the sync HWDGE queue overlap gpsimd loads.
 - scores: BD window + diagonal-shift DMA accumulate (rel-shift), one exp,
   DMA-transposed into pT [klen-part, q]; PV uses a ones-column for the
   softmax denominator.
 - o_proj and FFN2 emit natural-layout outputs directly (lhsT = attnT/hT
   slices, rhs = natural weight rows) -- no PE transposes on the way out.
 - ReduceScatter runs in bf16.
"""

import contextlib
import math

import numpy as np

import concourse.bass as bass
import concourse.bacc as bacc
import concourse.mybir as mybir
import concourse.tile as tile

F32 = mybir.dt.float32
BF16 = mybir.dt.bfloat16
AF = mybir.ActivationFunctionType
ALU = mybir.AluOpType


class Cfg:
    D = 1024      # model dim
    NHC = 8       # heads per core
    DH = 64       # head dim
    KL = 2048     # key length
    Q = 1024      # query length
    DI = 4096     # ffn inner
    LN_EPS = 1e-5
    N_CORES = 8

    HD = property(lambda s: s.NHC * s.DH)       # head dims per core (512)
    SCALE = property(lambda s: 1.0 / (s.DH ** 0.5))
    M = property(lambda s: s.KL - s.Q)          # mem length
    NS = property(lambda s: s.Q // 128)         # q tiles (8)
    DPT = property(lambda s: s.D // 128)        # 8
    HPT = property(lambda s: s.HD // 128)       # 4
    NTT = property(lambda s: s.KL // 128)       # 16
    WBW = property(lambda s: s.KL + 128)        # bdw buffer width (2176)
    TOKF = property(lambda s: s.Q // 2)         # ffn tokens per core (512)

    def jm(self, s):
        """exact key extent for q-tile s: multiple of 128."""
        return 128 * (s + 9)



def _mm512(nc, ps, lhsT, rhs_fn, width, start, stop):
    """Issue width//512 matmuls of <=512 cols into ps[:, off:off+...]."""
    for nb in range(0, width, 512):
        ne = min(width, nb + 512)
        nc.tensor.matmul(ps[:, nb:ne], lhsT, rhs_fn(nb, ne),
                         start=start, stop=stop)

def ts(i, n):
    return slice(i * n, (i + 1) * n)


def build_kernel(c: Cfg = None, collective=True, repeat=1):
    c = c or Cfg()
    nc = bacc.Bacc("TRN2", target_bir_lowering=False)

    io = {}
    def din(name, shape):
        io[name] = nc.dram_tensor(name, shape, F32, kind="ExternalInput")
    din("xw", [c.KL, c.D])
    din("r_in", [c.KL, c.D])
    din("qkvw", [c.D, 3 * c.HD])
    din("rnetw", [c.D, c.HD])
    din("oww", [c.HD, c.D])
    din("rwb", [1, c.HD])
    din("rrb", [1, c.HD])
    din("ln1g", [1, c.D]); din("ln1b", [1, c.D])
    din("ln2g", [1, c.D]); din("ln2b", [1, c.D])
    din("ffw1", [c.D, c.DI]); din("ffb1", [1, c.DI])
    din("ffw2", [c.DI, c.D]); din("ffb2", [1, c.D])
    din("wres", [c.TOKF, c.D])
    io["out"] = nc.dram_tensor("out", [c.TOKF, c.D], F32, kind="ExternalOutput")
    io["rs_bin"] = nc.dram_tensor("rs_bin", [c.Q, c.D], BF16)
    io["rs_bout"] = nc.dram_tensor("rs_bout", [c.TOKF, c.D], BF16)

    with tile.TileContext(nc) as tc:
        for _ in range(repeat):
            _body(tc, nc, c, io, collective=collective)
    nc.finalize()
    return nc


def _body(tc, nc, c, io, collective=True):
    ctx = contextlib.ExitStack()
    rg = [[i, i + 1] for i in range(0, c.N_CORES, 2)]
    with ctx:
        small = ctx.enter_context(tc.tile_pool(name="small", bufs=2))
        keep = ctx.enter_context(tc.tile_pool(name="keep", bufs=1))

        # manual pools, allocated in reverse order of release (LIFO stack)
        psA = tc.alloc_tile_pool(name="psA", bufs=2, space="PSUM")
        psD = tc.alloc_tile_pool(name="psD", bufs=2, space="PSUM")
        psV = tc.alloc_tile_pool(name="psV", bufs=2, space="PSUM")
        atp = tc.alloc_tile_pool(name="atp", bufs=1)
        owp = tc.alloc_tile_pool(name="owp", bufs=1)
        attk = tc.alloc_tile_pool(name="attk", bufs=1)
        phX = tc.alloc_tile_pool(name="phX", bufs=1)
        wqp = tc.alloc_tile_pool(name="wqp", bufs=1)
        phR = tc.alloc_tile_pool(name="phR", bufs=1)
        wrp = tc.alloc_tile_pool(name="wrp", bufs=1)

        # ---- persistent small constants ----
        rwb_s = keep.tile([128, c.HPT], F32, tag="rwb")
        rrb_s = keep.tile([128, c.HPT], F32, tag="rrb")
        nc.sync.dma_start(out=rwb_s[:], in_=bass.AP(
            tensor=io["rwb"].ap().tensor, offset=0, ap=[[1, 128], [128, c.HPT]]))
        nc.sync.dma_start(out=rrb_s[:], in_=bass.AP(
            tensor=io["rrb"].ap().tensor, offset=0, ap=[[1, 128], [128, c.HPT]]))
        # fold the attention scale into the biases (applied at Q^T creation)
        nc.vector.tensor_scalar_mul(out=rwb_s[:], in0=rwb_s[:],
                                    scalar1=float(c.SCALE))
        nc.vector.tensor_scalar_mul(out=rrb_s[:], in0=rrb_s[:],
                                    scalar1=float(c.SCALE))

        # ============ phase 1: weights + r/x transposed loads ============
        # weight loads ride the sync HWDGE queue + DVE casts, so the Pool
        # (SWDGE) engine only paces the 32 casted r/x row-tile loads.
        def mk_wload(wstage):
            def wload(pool, src, p, width, tag):
                f32t = wstage.tile([128, width], F32, tag="wld")
                nc.sync.dma_start(out=f32t[:], in_=src[ts(p, 128), :])
                t = pool.tile([128, width], BF16, tag="%s_%d" % (tag, p))
                nc.vector.tensor_copy(out=t[:], in_=f32t[:])
                return t
            return wload

        if True:
            with tc.tile_pool(name="wst", bufs=2) as wstage:
                wload = mk_wload(wstage)
                wr_t = [wload(wrp, io["rnetw"], p, c.HD, "wr")
                        for p in range(c.DPT)]

            rT = phR.tile([128, c.DPT * c.KL], BF16, tag="rT")

            def load_transposed(src, dst, stage, via_act=False, tiles=None):
                for tt in (tiles if tiles is not None else range(c.NTT)):
                    if via_act:
                        # scalar-queue f32 load + Act cast: keeps the Pool
                        # (SWDGE) queue free for the r pipeline
                        ft = stage.tile([128, c.D], F32, tag="ldf")
                        nc.scalar.dma_start(out=ft[:], in_=src[ts(tt, 128), :])
                        bt = stage.tile([128, c.D], BF16, tag="ld")
                        nc.scalar.activation(out=bt[:], in_=ft[:], func=AF.Copy)
                    else:
                        bt = stage.tile([128, c.D], BF16, tag="ld")
                        nc.gpsimd.dma_start(out=bt[:], in_=src[ts(tt, 128), :])
                    dstap = bass.AP(
                        tensor=dst.tensor, offset=dst.offset + tt * 128,
                        ap=[[c.DPT * c.KL, 128], [c.KL, c.DPT], [1, 128]])
                    nc.sync.dma_start(out=dstap, in_=bt[:], transpose=True)

            xT = phX.tile([128, c.DPT * c.KL], BF16, tag="xT")
            with tc.tile_pool(name="stR", bufs=4) as stage, \
                 tc.tile_pool(name="stX", bufs=2) as stageX:
                # r first (rTp is the first PE consumer); x overlaps on
                # the scalar/Act queues
                load_transposed(io["r_in"], rT, stage)
                load_transposed(io["xw"], xT, stageX, via_act=True)

            with tc.tile_pool(name="wst2", bufs=2) as wstage2:
                wload2 = mk_wload(wstage2)
                qkv_t = [wload2(wqp, io["qkvw"], p, 3 * c.HD, "qkv")
                         for p in range(c.DPT)]



        ow_t = []
        for p in range(c.HPT):
            t = owp.tile([128, c.D], BF16, tag="ow_%d" % p)
            nc.gpsimd.dma_start(out=t[:], in_=io["oww"][ts(p, 128), :])
            ow_t.append(t)

        # ============ phase 2: projections ============
        rTp = attk.tile([128, c.HPT * c.KL], BF16, tag="rTp")
        kT = attk.tile([128, c.HPT * c.KL], BF16, tag="kT")
        VW = c.NHC * 65
        vb = attk.tile([128, c.NTT * VW], BF16, tag="vb")
        rwq = attk.tile([128, c.HPT * c.Q], BF16, tag="rwq")
        rrq = attk.tile([128, c.HPT * c.Q], BF16, tag="rrq")
        attnT = atp.tile([128, c.HPT * c.Q], BF16, tag="attnT")

        # rTp = (r @ r_net_w)^T  [hd-part, klen]
        for m in range(c.HPT):
            for ch in range(2):
                ps = psA.tile([128, 1024], F32, tag="a")
                for k in range(c.DPT):
                    _mm512(nc, ps, wr_t[k][:, ts(m, 128)],
                           lambda nb, ne, k=k: rT[:, k * c.KL + ch * 1024 + nb:
                                                  k * c.KL + ch * 1024 + ne],
                           1024, start=(k == 0), stop=(k == c.DPT - 1))
                nc.scalar.activation(
                    out=rTp[:, m * c.KL + ch * 1024: m * c.KL + (ch + 1) * 1024],
                    in_=ps[:], func=AF.Copy)
        wrp.release()
        phR.release()

        # K^T [hd-part, klen]
        for m in range(c.HPT):
            for ch in range(2):
                ps = psA.tile([128, 1024], F32, tag="a")
                for k in range(c.DPT):
                    _mm512(nc, ps, qkv_t[k][:, c.HD + m * 128: c.HD + (m + 1) * 128],
                           lambda nb, ne, k=k: xT[:, k * c.KL + ch * 1024 + nb:
                                                  k * c.KL + ch * 1024 + ne],
                           1024, start=(k == 0), stop=(k == c.DPT - 1))
                nc.scalar.activation(
                    out=kT[:, m * c.KL + ch * 1024: m * c.KL + (ch + 1) * 1024],
                    in_=ps[:], func=AF.Copy)
        # V natural [klen-part, hd] (+ ones col per head for softmax denom)
        for jt in range(c.NTT):
            ps = psV.tile([128, c.HD], F32, tag="v")
            for k in range(c.DPT):
                nc.tensor.matmul(
                    ps[:], xT[:, k * c.KL + jt * 128: k * c.KL + (jt + 1) * 128],
                    qkv_t[k][:, 2 * c.HD: 3 * c.HD],
                    start=(k == 0), stop=(k == c.DPT - 1))
            dst = bass.AP(
                tensor=vb.tensor, offset=vb.offset + jt * VW,
                ap=[[c.NTT * VW, 128], [65, c.NHC], [1, c.DH]])
            nc.vector.tensor_copy(out=dst, in_=ps[:])
            ones = bass.AP(
                tensor=vb.tensor, offset=vb.offset + jt * VW + c.DH,
                ap=[[c.NTT * VW, 128], [65, c.NHC], [1, 1]])
            nc.vector.memset(ones, 1.0)
        # Q^T with scale and biases folded: rwq = SCALE*q + SCALE*rwb etc.
        for m in range(c.HPT):
            ps = psA.tile([128, 1024], F32, tag="a")
            for k in range(c.DPT):
                _mm512(nc, ps, qkv_t[k][:, ts(m, 128)],
                       lambda nb, ne, k=k: xT[:, k * c.KL + c.M + nb:
                                              k * c.KL + c.M + ne],
                       1024, start=(k == 0), stop=(k == c.DPT - 1))
            sl = ts(m, c.Q)
            nc.scalar.activation(out=rwq[:, sl], in_=ps[:],
                                 func=AF.Identity, bias=rwb_s[:, m:m + 1],
                                 scale=float(c.SCALE))
            nc.vector.tensor_scalar(out=rrq[:, sl], in0=ps[:],
                                    scalar1=rrb_s[:, m:m + 1],
                                    scalar2=float(c.SCALE),
                                    op0=ALU.mult, op1=ALU.add)
        wqp.release()
        phX.release()

        # ============ phase 3: attention ============
        # software-pipelined: PV of head h-1 is issued after the scores of
        # head h, so the in-order PE queue never stalls on pT completion.
        with tc.tile_pool(name="scoreA", bufs=3) as scoreA, \
             tc.tile_pool(name="scoreB", bufs=5) as scoreB, \
             tc.tile_pool(name="ptp", bufs=2) as ptp:

            pend = []  # skewed (sb, pT, s) exp+transpose stages
            cchn = [0]   # rotating copy-engine counter

            def copy_chunk(dst, src, eng):
                if eng == "act":
                    nc.scalar.activation(out=dst, in_=src, func=AF.Copy)
                else:
                    nc.vector.tensor_copy(out=dst, in_=src)

            ROT = ("dve", "act")

            def flush_one():
                if not pend:
                    return
                sb, pT, s = pend.pop(0)
                jmx = c.jm(s)
                pb = scoreB.tile([128, c.KL], BF16, tag="pb", name="pb")
                nc.scalar.activation(out=pb[:, 0:jmx], in_=sb[:, 0:jmx],
                                     func=AF.Exp)
                dstap = bass.AP(
                    tensor=pT.tensor, offset=pT.offset + s * 128,
                    ap=[[c.NTT * c.Q, 128], [c.Q, jmx // 128], [1, 128]])
                nc.sync.dma_start(out=dstap, in_=pb[:, 0:jmx], transpose=True)

            def flush_head(pT_old):
                while pend and pend[0][1] is pT_old:
                    flush_one()

            def score_iter(h, s, pT):
                hp, hr = h // 2, (h % 2) * 64
                if True:
                    jmx = c.jm(s)
                    wst = c.Q - 128 * (s + 1)
                    # BD window [128, jmx] + masked tail
                    bdw = scoreA.tile([128, c.WBW], BF16, tag="bdw")
                    for lo in range(0, jmx, 512):
                        hi = min(jmx, lo + 512)
                        ps = psD.tile([128, 512], F32, tag="d")
                        nc.tensor.matmul(
                            ps[:, 0:hi - lo],
                            rrq[hr:hr + 64, hp * c.Q + s * 128: hp * c.Q + (s + 1) * 128],
                            rTp[hr:hr + 64, hp * c.KL + wst + lo: hp * c.KL + wst + hi],
                            start=True, stop=True)
                        copy_chunk(bdw[:, lo:hi], ps[:, 0:hi - lo],
                                   ROT[cchn[0] % 2]); cchn[0] += 1
                    nc.gpsimd.memset(bdw[:, jmx: jmx + 128], -30.0)
                    # AC scores [128, jmx]
                    sb = scoreB.tile([128, c.KL], BF16, tag="sb")
                    for lo in range(0, jmx, 1024):
                        hi = min(jmx, lo + 1024)
                        ps = psA.tile([128, 1024], F32, tag="a")
                        _mm512(nc, ps,
                               rwq[hr:hr + 64, hp * c.Q + s * 128: hp * c.Q + (s + 1) * 128],
                               lambda nb, ne: kT[hr:hr + 64, hp * c.KL + lo + nb:
                                                 hp * c.KL + lo + ne],
                               hi - lo, start=True, stop=True)
                        copy_chunk(sb[:, lo:hi], ps[:, 0:hi - lo],
                                   ROT[cchn[0] % 2])
                        cchn[0] += 1
                    # rel-shift: S[p, j] += BD[p, 127 - p + j]
                    diag = bass.AP(tensor=bdw.tensor, offset=bdw.offset + 127,
                                   ap=[[c.WBW - 1, 128], [1, jmx]])
                    nc.gpsimd.dma_start(out=sb[:, 0:jmx], in_=diag,
                                        accum_op=ALU.add)
                    pend.append((sb, pT, s))
                    if len(pend) > 4:
                        flush_one()

            def issue_pv(h, pT, chunks=(0, 1)):
                hp, hr = h // 2, (h % 2) * 64
                for c2 in chunks:
                    lo, hi = c2 * 512, (c2 + 1) * 512
                    njt = 12 if c2 == 0 else 16
                    ps = psV.tile([65, 512], F32, tag="v")
                    for jt in range(njt):
                        nlo = max(lo, 128 * (jt - 8))
                        nc.tensor.matmul(
                            ps[0:65, nlo - lo:512],
                            vb[:, jt * VW + h * 65: jt * VW + h * 65 + 65],
                            pT[:, jt * c.Q + nlo: jt * c.Q + hi],
                            start=(jt == 0), stop=(jt == njt - 1))
                    rd = small.tile([1, 512], F32, tag="rd")
                    nc.vector.reciprocal(out=rd[:], in_=ps[64:65, :])
                    rdb = small.tile([64, 512], F32, tag="rdb")
                    src_b = bass.AP(tensor=rd.tensor, offset=rd.offset,
                                    ap=[[512, 1], [0, 64], [1, 512]])
                    nc.sync.dma_start(out=rdb[:], in_=src_b)
                    nc.vector.tensor_tensor(
                        out=attnT[hr:hr + 64, hp * c.Q + lo: hp * c.Q + hi],
                        in0=ps[0:64, :], in1=rdb[:], op=ALU.mult)

            prev = None  # (hA, pT_A, hB, pT_B) of previous pair
            for hp in range(c.NHC // 2):
                hA, hB = 2 * hp, 2 * hp + 1
                if prev is not None:
                    pA, ptA, pB, ptB = prev
                    while pend:
                        flush_one()
                    issue_pv(pA, ptA)
                pT_A = ptp.tile([128, c.NTT * c.Q], BF16, tag="pT", name="pTA")
                if prev is not None:
                    issue_pv(pB, ptB)
                pT_B = ptp.tile([128, c.NTT * c.Q], BF16, tag="pT", name="pTB")
                for s in range(c.NS):
                    score_iter(hA, s, pT_A)
                    score_iter(hB, s, pT_B)
                prev = (hA, pT_A, hB, pT_B)
            while pend:
                flush_one()
            pA, ptA, pB, ptB = prev
            issue_pv(pA, ptA)
            issue_pv(pB, ptB)

        attk.release()

        # ============ phase 4: o_proj (natural out) -> ReduceScatter ============
        with tc.tile_pool(name="stO", bufs=3) as stage:
            for qb in range(c.NS):
                ost = stage.tile([128, c.D], BF16, tag="ost")
                for half in range(2):
                    ps = psV.tile([128, 512], F32, tag="v")
                    for k in range(c.HPT):
                        nc.tensor.matmul(
                            ps[:], attnT[:, k * c.Q + qb * 128: k * c.Q + (qb + 1) * 128],
                            ow_t[k][:, half * 512: (half + 1) * 512],
                            start=(k == 0), stop=(k == c.HPT - 1))
                    nc.scalar.activation(out=ost[:, half * 512:(half + 1) * 512],
                                         in_=ps[:], func=AF.Copy)
                nc.sync.dma_start(out=io["rs_bin"][ts(qb, 128), :], in_=ost[:])
        owp.release()
        atp.release()
        psV.release()
        psD.release()
        psA.release()

        # ============ phase 5: LN1 + FFN + LN2 ============
        # w1 loads first (DMA only, overlap o_proj/collective), then consts,
        # then the collective, then everything that depends on it.
        w1p = ctx.enter_context(tc.tile_pool(name="w1p", bufs=1))
        w1_t = []
        for k in range(c.DPT):
            t = w1p.tile([128, c.DI], BF16, tag="w1_%d" % k)
            nc.gpsimd.dma_start(out=t[:], in_=io["ffw1"][ts(k, 128), :])
            w1_t.append(t)

        phE = ctx.enter_context(tc.tile_pool(name="phE", bufs=1))
        eps_t = phE.tile([128, 1], F32, tag="eps")
        nc.vector.memset(eps_t[:], c.LN_EPS)
        lns = {}
        for nm in ("ln1g", "ln1b", "ln2g", "ln2b"):
            tl = phE.tile([128, c.D], F32, tag=nm)
            bcast = bass.AP(tensor=io[nm].ap().tensor, offset=0,
                            ap=[[0, 128], [1, c.D]])
            nc.sync.dma_start(out=tl[:], in_=bcast)
            lns[nm] = tl
        fb1 = phE.tile([128, c.DI // 128], F32, tag="fb1")
        nc.sync.dma_start(out=fb1[:], in_=bass.AP(
            tensor=io["ffb1"].ap().tensor, offset=0, ap=[[1, 128], [128, c.DI // 128]]))
        fb2n = phE.tile([128, c.D], F32, tag="fb2n")
        nc.sync.dma_start(out=fb2n[:], in_=bass.AP(
            tensor=io["ffb2"].ap().tensor, offset=0, ap=[[0, 128], [1, c.D]]))

        ntt = c.TOKF // 128  # 4
        ffn = ctx.enter_context(tc.tile_pool(name="ffn", bufs=1))
        ln1r = ffn.tile([128, ntt * c.D], BF16, tag="ln1r")  # ln1 out + b2
        lnT = ffn.tile([128, c.DPT * c.TOKF], BF16, tag="lnT")
        hT = ffn.tile([128, (c.DI // 128) * c.TOKF], BF16, tag="hT")
        wres4 = ffn.tile([128, ntt * c.D], BF16, tag="wres4")
        for tt in range(ntt):
            nc.gpsimd.dma_start(out=wres4[:, ts(tt, c.D)],
                                in_=io["wres"][ts(tt, 128), :])

        if collective:
            nc.gpsimd.collective_compute(
                "ReduceScatter", ALU.add, replica_groups=rg,
                ins=[io["rs_bin"].ap().opt()], outs=[io["rs_bout"].ap().opt()])
        else:
            nc.sync.dma_start(out=io["rs_bout"].ap().opt(),
                              in_=io["rs_bin"].ap()[0:c.TOKF, :].opt())

        # 8 single-bank accumulators for the k-outer FFN2 (and FFN1/LN use)
        psF = ctx.enter_context(tc.tile_pool(name="psF", bufs=1, space="PSUM"))

        def psf(i, shape, dtype=F32):
            return psF.tile(shape, dtype, tag="p%d" % (i % 8),
                            name="psf%d" % (i % 8))

        with tc.tile_pool(name="stE", bufs=2) as stage, \
             tc.tile_pool(name="w2s", bufs=8) as w2s:
            for tt in range(ntt):
                zb = stage.tile([128, c.D], BF16, tag="zb")
                nc.sync.dma_start(out=zb[:], in_=io["rs_bout"][ts(tt, 128), :])
                z = stage.tile([128, c.D], F32, tag="z")
                nc.vector.tensor_tensor(out=z[:], in0=wres4[:, ts(tt, c.D)],
                                        in1=zb[:], op=ALU.add)
                lsl = slice(tt * c.D, (tt + 1) * c.D)
                _layernorm_nat(nc, c, small, z[:], eps_t,
                               lns["ln1g"], lns["ln1b"], ln1r[:, lsl])
                znb = stage.tile([128, c.D], BF16, tag="znb")
                nc.vector.tensor_copy(out=znb[:], in_=ln1r[:, lsl])
                dstap = bass.AP(
                    tensor=lnT.tensor, offset=lnT.offset + tt * 128,
                    ap=[[c.DPT * c.TOKF, 128], [c.TOKF, c.DPT], [1, 128]])
                nc.sync.dma_start(out=dstap, in_=znb[:], transpose=True)
                # pre-add b2 for the FFN2 residual
                nc.vector.tensor_tensor(out=ln1r[:, lsl], in0=ln1r[:, lsl],
                                        in1=fb2n[:], op=ALU.add)
            # FFN1: hT[di, tok], m-outer with resident w1; token-halves so
            # the first half starts after only 2 of 4 LN1 tiles
            for g in range(2):
                for m in range(c.DI // 128):
                    ps = psf(m, [128, 256])
                    for k in range(c.DPT):
                        nc.tensor.matmul(
                            ps[:], w1_t[k][:, ts(m, 128)],
                            lnT[:, k * c.TOKF + g * 256: k * c.TOKF + (g + 1) * 256],
                            start=(k == 0), stop=(k == c.DPT - 1))
                    nc.scalar.activation(
                        out=hT[:, m * c.TOKF + g * 256: m * c.TOKF + (g + 1) * 256],
                        in_=ps[:], func=AF.Relu, bias=fb1[:, m:m + 1])
            # FFN2: k-outer, streaming w2, natural out [tok, d]
            nkt = c.DI // 128
            acc = [psf(i, [128, 512]) for i in range(8)]
            for k in range(nkt):
                w2t = w2s.tile([128, c.D], BF16, tag="w2")
                nc.gpsimd.dma_start(out=w2t[:], in_=io["ffw2"][ts(k, 128), :])
                for tb in range(ntt):
                    for half in range(2):
                        nc.tensor.matmul(
                            acc[tb * 2 + half][:],
                            hT[:, k * c.TOKF + tb * 128: k * c.TOKF + (tb + 1) * 128],
                            w2t[:, half * 512:(half + 1) * 512],
                            start=(k == 0), stop=(k == nkt - 1))
            for tb in range(ntt):
                o2n = stage.tile([128, c.D], F32, tag="o2n")
                for half in range(2):
                    nc.vector.tensor_tensor(
                        out=o2n[:, half * 512:(half + 1) * 512],
                        in0=acc[tb * 2 + half][:],
                        in1=ln1r[:, tb * c.D + half * 512: tb * c.D + (half + 1) * 512],
                        op=ALU.add)
                fin = stage.tile([128, c.D], F32, tag="fin")
                _layernorm_nat(nc, c, small, o2n[:], eps_t,
                               lns["ln2g"], lns["ln2b"], fin[:])
                nc.sync.dma_start(out=io["out"][ts(tb, 128), :], in_=fin[:])


def _layernorm_nat(nc, c, small, z, eps_t, g, b, out_dst):
    """LayerNorm over the free axis of z [128, D] fp32."""
    BN_FMAX = nc.vector.BN_STATS_FMAX
    d = z.shape[-1]
    sub = math.gcd(BN_FMAX, d)
    nsub = d // sub
    zr = z.rearrange("p (n f) -> p n f", f=sub)
    stats = small.tile([128, nsub, nc.vector.BN_STATS_DIM], F32, tag="bnst")
    for i in range(nsub):
        nc.vector.bn_stats(out=stats[:, i, :], in_=zr[:, i, :])
    mv = small.tile([128, nc.vector.BN_AGGR_DIM], F32, tag="bnag")
    nc.vector.bn_aggr(out=mv[:], in_=stats[:])
    mean, var = mv[:, 0:1], mv[:, 1:2]
    nc.scalar.activation(out=var, in_=var, func=AF.Sqrt, bias=eps_t[:], scale=1.0)
    nc.vector.reciprocal(out=var, in_=var)
    nc.vector.tensor_scalar(out=out_dst, in0=z, scalar1=mean, scalar2=var,
                            op0=ALU.subtract, op1=ALU.mult)
    nc.vector.tensor_tensor(out=out_dst, in0=out_dst, in1=g[:, 0:d], op=ALU.mult)
    nc.vector.tensor_tensor(out=out_dst, in0=out_dst, in1=b[:, 0:d], op=ALU.add)


# ============================================================
# host-side sharding + entry point
# ============================================================

def shard_inputs(inputs, c: Cfg = None):
    c = c or Cfg()
    w = np.asarray(inputs["w"], np.float32)
    r = np.asarray(inputs["r"], np.float32)
    mems = np.asarray(inputs["mems"], np.float32)
    qkv_w = np.asarray(inputs["qkv_w"], np.float32)
    r_net_w = np.asarray(inputs["r_net_w"], np.float32)
    o_w = np.asarray(inputs["o_w"], np.float32)
    r_w_bias = np.asarray(inputs["r_w_bias"], np.float32).reshape(-1)
    r_r_bias = np.asarray(inputs["r_r_bias"], np.float32).reshape(-1)
    NHD = qkv_w.shape[1] // 3
    in_maps = []
    for core in range(c.N_CORES):
        b, hh = core // 2, core % 2
        hsl = slice(hh * c.HD, (hh + 1) * c.HD)
        xw_c = np.concatenate([mems[:, b, :], w[:, b, :]], axis=0)
        qkvw_c = np.concatenate([qkv_w[:, j * NHD + hh * c.HD:
                                       j * NHD + (hh + 1) * c.HD]
                                 for j in range(3)], axis=1)
        in_maps.append({
            "xw": np.ascontiguousarray(xw_c),
            "r_in": np.ascontiguousarray(r[:, 0, :]),
            "qkvw": np.ascontiguousarray(qkvw_c),
            "rnetw": np.ascontiguousarray(r_net_w[:, hsl]),
            "oww": np.ascontiguousarray(o_w[hsl, :]),
            "rwb": np.ascontiguousarray(r_w_bias[hsl][None, :]),
            "rrb": np.ascontiguousarray(r_r_bias[hsl][None, :]),
            "ln1g": np.asarray(inputs["ln1_g"], np.float32).reshape(1, -1),
            "ln1b": np.asarray(inputs["ln1_b"], np.float32).reshape(1, -1),
            "ln2g": np.asarray(inputs["ln2_g"], np.float32).reshape(1, -1),
            "ln2b": np.asarray(inputs["ln2_b"], np.float32).reshape(1, -1),
            "ffw1": np.asarray(inputs["ff_w1"], np.float32),
            "ffb1": np.asarray(inputs["ff_b1"], np.float32).reshape(1, -1),
            "ffw2": np.asarray(inputs["ff_w2"], np.float32),
            "ffb2": np.asarray(inputs["ff_b2"], np.float32).reshape(1, -1),
            "wres": np.ascontiguousarray(w[hh * c.TOKF:(hh + 1) * c.TOKF, b, :]),
        })
    return in_maps


def unshard_output(results, inputs, c: Cfg = None):
    c = c or Cfg()
    w = np.asarray(inputs["w"])
    Q, B, D = w.shape
    out = np.zeros((Q, B, D), np.float32)
    for core in range(c.N_CORES):
        b, hh = core // 2, core % 2
        out[hh * c.TOKF:(hh + 1) * c.TOKF, b, :] = results[core]["out"]
    return out


_NC_CACHE = {}


def kernel(**inputs):
    if "nc" not in _NC_CACHE:
        _NC_CACHE["nc"] = build_kernel()
    nc = _NC_CACHE["nc"]
    in_maps = shard_inputs(inputs)
    from concourse.bass_utils import run_bass_kernel_spmd
    res = run_bass_kernel_spmd(nc, in_maps, core_ids=list(range(Cfg.N_CORES)))
    return unshard_output(res.results, inputs)
